# revision 3
# baseline (speedup 1.0000x reference)
"""DotLoss kernel for Trainium2, data-parallel over 8 NeuronCores.

loss = mean_i[ relu(1 + dot(img[I[i]], aud[i]) - dot(img[i], aud[i]))
             + relu(1 + dot(img[i], aud[A[i]]) - dot(img[i], aud[i])) ]

Sharding: data-parallel over the batch axis; the host materializes the
impostor rows img[I[i]] / aud[A[i]] per shard while packing, so each
core consumes four aligned streams and the device kernel is pure
streaming.

v3: the binding resource is the DMA *write* side into SBUF (~400 GB/s
across the 16 SDMA engines), so the goal is fewer bytes landing in
SBUF, while every product still runs on DVE in bf16 2x mode:
  - local streams (li, la): fp8-e4m3 in HBM, one SWDGE cast-DMA per
    chunk widens them to bf16 on the way in (8.4MB written).
  - impostor streams (gi, ga): stay fp8 through the DMA (4.2MB
    written), widened to bf16 on-chip by engines with spare cycles:
    ga on ScalarE (activation Copy), gi split between GPSIMD
    (tensor_copy, a-blocks 0-1) and DVE (tensor_copy 2x_2p, a-blocks
    2-3).
  -> 12.6MB SBUF writes vs 16.8MB for all-bf16: ~31.5us stream floor.
randn data (|x| <= ~5.4) sits far inside TRN fp8_exp4's +-240 range;
measured end-to-end rel err ~1.3e-3 vs fp32 reference.

Compute per chunk:
  - DVE: 3 tensor_tensor bf16 multiplies (2x_1p) + 1 half-convert.
  - TensorE: sum over D via matmul with +/-ones stationary; PSUM X
    accumulates iimp-anchor, PSUM Y aimp-anchor.
  - ScalarE: hinge relu(1 + x) + row-sum via activation(accum_out)
    straight off PSUM.
Each core emits a [128,1] fp32 partial (all partitions identical);
the host reads row 0 per core, sums, divides by N.
"""

import numpy as np

N, D = 32768, 512
NCORES = 8
SHARD = N // NCORES          # 4096 rows per core
P = 128
A = D // P                   # 4 partition-blocks of D
CH = 512                     # rows per chunk
NCH = SHARD // CH            # 8 chunks
_CACHE = {}


def _build_nc():
    import concourse.bacc as bacc
    import concourse.mybir as mybir
    import concourse.tile as tile
    from contextlib import ExitStack

    fp32 = mybir.dt.float32
    bf16 = mybir.dt.bfloat16
    fp8 = mybir.dt.float8e4

    nc = bacc.Bacc("TRN2")
    # local streams (li, la): cast-DMA'd to bf16 on the way in
    dloc = nc.dram_tensor("dloc", [NCH, P, 2, A, CH], fp8,
                          kind="ExternalInput")
    # impostor streams (gi, ga): land fp8, widened on-chip
    dimp = nc.dram_tensor("dimp", [NCH, P, 2, A, CH], fp8,
                          kind="ExternalInput")
    onesc = nc.dram_tensor("onesc", [P, 2 * P], bf16, kind="ExternalInput")
    partial = nc.dram_tensor("partial", [P, 1], fp32, kind="ExternalOutput")

    mult = mybir.AluOpType.mult
    add = mybir.AluOpType.add
    relu = mybir.ActivationFunctionType.Relu
    copyf = mybir.ActivationFunctionType.Copy

    with ExitStack() as ctx:
        tc = ctx.enter_context(tile.TileContext(nc))
        iop = ctx.enter_context(tc.tile_pool(name="iop", bufs=4))
        ifp = ctx.enter_context(tc.tile_pool(name="ifp", bufs=4))
        cvp = ctx.enter_context(tc.tile_pool(name="cvp", bufs=4))
        prp = ctx.enter_context(tc.tile_pool(name="prp", bufs=4))
        psp = ctx.enter_context(tc.psum_pool(name="psp", bufs=4))
        hxp = ctx.enter_context(tc.tile_pool(name="hxp", bufs=4))
        acc = ctx.enter_context(tc.tile_pool(name="acc", bufs=1))

        ones_sb = acc.tile([P, 2 * P], bf16, tag="ones")
        nc.sync.dma_start(out=ones_sb[:], in_=onesc[:])
        pos = ones_sb[:, 0:P]
        neg = ones_sb[:, P:2 * P]

        hsum = acc.tile([P, 2 * NCH], fp32, tag="hsum")

        for k in range(NCH):
            loc = iop.tile([P, 2, A, CH], bf16, tag="loc")
            nc.gpsimd.dma_start(out=loc[:], in_=dloc[k])   # fp8 -> bf16 cast
            imp8 = ifp.tile([P, 2, A, CH], fp8, tag="imp8")
            nc.sync.dma_start(out=imp8[:], in_=dimp[k])    # raw fp8
            li = loc[:, 0]
            la = loc[:, 1]

            imp = cvp.tile([P, 2, A, CH], bf16, tag="imp")
            gi = imp[:, 0]
            ga = imp[:, 1]
            # widen gi: a-blocks 0-1 on GPSIMD, 2-3 on DVE (2x_2p)
            nc.gpsimd.tensor_copy(out=gi[:, 0:2], in_=imp8[:, 0, 0:2])
            nc.vector.tensor_copy(out=gi[:, 2:4], in_=imp8[:, 0, 2:4])
            # widen ga on ScalarE
            nc.scalar.activation(out=ga[:], in_=imp8[:, 1], func=copyf)

            prA = prp.tile([P, A, CH], bf16, tag="prA")
            nc.vector.tensor_tensor(out=prA[:], in0=li[:], in1=la[:], op=mult)
            prI = prp.tile([P, A, CH], bf16, tag="prI")
            nc.vector.tensor_tensor(out=prI[:], in0=gi[:], in1=la[:], op=mult)
            prU = prp.tile([P, A, CH], bf16, tag="prU")
            nc.vector.tensor_tensor(out=prU[:], in0=li[:], in1=ga[:], op=mult)

            px = psp.tile([P, CH], fp32, tag="px")
            py = psp.tile([P, CH], fp32, tag="py")
            for a in range(A):
                nc.tensor.matmul(px[:], pos, prI[:, a], start=(a == 0),
                                 stop=False, skip_group_check=True)
            for a in range(A):
                nc.tensor.matmul(py[:], pos, prU[:, a], start=(a == 0),
                                 stop=False, skip_group_check=True)
            for a in range(A):
                nc.tensor.matmul(px[:], neg, prA[:, a], start=False,
                                 stop=(a == A - 1), skip_group_check=True)
            for a in range(A):
                nc.tensor.matmul(py[:], neg, prA[:, a], start=False,
                                 stop=(a == A - 1), skip_group_check=True)

            hx = hxp.tile([P, CH], bf16, tag="hx")
            nc.scalar.activation(out=hx[:], in_=px[:], func=relu, bias=1.0,
                                 scale=1.0, accum_out=hsum[:, 2 * k:2 * k + 1])
            hy = hxp.tile([P, CH], bf16, tag="hy")
            nc.scalar.activation(out=hy[:], in_=py[:], func=relu, bias=1.0,
                                 scale=1.0,
                                 accum_out=hsum[:, 2 * k + 1:2 * k + 2])

        psum_t = acc.tile([P, 1], fp32, tag="psum")
        nc.vector.tensor_reduce(
            out=psum_t[:], in_=hsum[:], axis=mybir.AxisListType.X, op=add,
        )
        nc.sync.dma_start(out=partial[:], in_=psum_t[:])

    nc.compile()
    return nc


def _get_nc():
    if "nc" not in _CACHE:
        _CACHE["nc"] = _build_nc()
    return _CACHE["nc"]


def _block(xt):
    """[D, SHARD] -> [NCH, P, A, CH]: per (chunk, partition) contiguous."""
    return np.ascontiguousarray(
        xt.reshape(A, P, NCH, CH).transpose(2, 1, 0, 3))


def make_in_maps(image_outputs, audio_outputs, I_imp_ind, A_imp_ind):
    import ml_dtypes

    bf16 = np.dtype(ml_dtypes.bfloat16)
    fp8 = np.dtype(ml_dtypes.float8_e4m3fn)
    img = np.asarray(image_outputs, dtype=np.float32)
    aud = np.asarray(audio_outputs, dtype=np.float32)
    I_imp = np.asarray(I_imp_ind).astype(np.int64)
    A_imp = np.asarray(A_imp_ind).astype(np.int64)
    ones = np.concatenate(
        [np.ones((P, P), np.float32), -np.ones((P, P), np.float32)],
        axis=1).astype(bf16)
    in_maps = []
    for c in range(NCORES):
        base = c * SHARD
        sl = slice(base, base + SHARD)
        loc = np.stack([_block(img[sl].T), _block(aud[sl].T)], axis=2)
        imp = np.stack([_block(img[I_imp[sl]].T),
                        _block(aud[A_imp[sl]].T)], axis=2)
        in_maps.append({
            "dloc": np.ascontiguousarray(loc).astype(fp8),
            "dimp": np.ascontiguousarray(imp).astype(fp8),
            "onesc": ones,
        })
    return in_maps


def kernel(image_outputs, audio_outputs, I_imp_ind, A_imp_ind):
    from concourse import bass_utils

    nc = _get_nc()
    in_maps = make_in_maps(image_outputs, audio_outputs, I_imp_ind, A_imp_ind)
    res = bass_utils.run_bass_kernel_spmd(nc, in_maps, list(range(NCORES))).results
    # every PSUM partition holds identical broadcast sums -> use row 0 only
    total = sum(float(r["partial"][0, 0]) for r in res)
    return np.float32(total / N)


# revision 6
# speedup vs baseline: 1.2897x; 1.2897x over previous
"""DotLoss kernel for Trainium2, data-parallel over 8 NeuronCores.

loss = mean_i[ relu(1 + dot(img[I[i]], aud[i]) - dot(img[i], aud[i]))
             + relu(1 + dot(img[i], aud[A[i]]) - dot(img[i], aud[i])) ]

Sharding: data-parallel over the batch axis; the host materializes the
impostor rows img[I[i]] / aud[A[i]] per shard while packing, so each
core consumes four aligned streams and the device kernel is pure
streaming.

v4: the binding resource is the DMA *write* side into SBUF (~400 GB/s
aggregate over the 16 SDMA engines -> 42us for 16.8MB all-bf16), so
SBUF-landing bytes are minimized while every product still runs on
DVE in bf16 2x mode.  All HBM payloads are fp8-e4m3 (randn |x|<=5.4
sits far inside TRN fp8_exp4's +-240; measured rel err ~1.3e-3).
Per chunk two DMAs:
  - cast-DMA (SWDGE fp8->bf16): li (4 a-blks), la (4), gi a-blks 0-1
    -> 10KB/partition bf16 written.
  - raw-DMA (HWDGE fp8): gi a-blks 2-3 + ga (4) -> 3KB/partition.
  -> 13.6MB SBUF writes (vs 16.8 all-bf16): ~34us stream window.
On-chip widening of the fp8 remainder uses spare engine cycles:
  - ga (2048 elems/part): ScalarE activation-Copy, 2.0us/chunk
    (ScalarE total with hinges ~30us < window).
  - gi a-blks 2-3 (1024): DVE tensor_copy in 2x_2p mode, 0.6us/chunk
    (DVE total with products ~32us < window).
GPSIMD is kept OFF the datapath: its software CAST ran at ~3.5ns/elem
and its SBUF traffic knocked concurrent DVE ops off their fast mode.

Compute per chunk: DVE tensor_tensor products (prA=li*la, prI in two
halves, prU=li*ga); TensorE reduces over D via matmuls with a +/-ones
stationary (PSUM X = iimp-anchor, PSUM Y = aimp-anchor); ScalarE
computes relu(1+x) + row-sum in one activation(accum_out) off PSUM.
The last chunk's compute runs in two column-halves to halve the
serial tail after the final DMA.  Each core emits a [128,1] fp32
partial (all partitions identical); host sums row 0 over cores / N.
"""

import numpy as np

N, D = 32768, 512
NCORES = 8
SHARD = N // NCORES          # 4096 rows per core
P = 128
A = D // P                   # 4 partition-blocks of D
CH = 512                     # rows per chunk
NCH = SHARD // CH            # 8 chunks
_CACHE = {}


def _build_nc():
    import concourse.bacc as bacc
    import concourse.mybir as mybir
    import concourse.tile as tile
    from contextlib import ExitStack

    fp32 = mybir.dt.float32
    bf16 = mybir.dt.bfloat16
    fp8 = mybir.dt.float8e4

    nc = bacc.Bacc("TRN2")
    # cast payload: [li a0-3 | la a0-3 | gi a0-1] -> 10 slots of CH
    dcast = nc.dram_tensor("dcast", [NCH, P, 10, CH], fp8,
                           kind="ExternalInput")
    # raw payload: [gi a2-3 | ga a0-3] -> 6 slots of CH
    draw = nc.dram_tensor("draw", [NCH, P, 6, CH], fp8,
                          kind="ExternalInput")
    onesc = nc.dram_tensor("onesc", [P, 2 * P], bf16, kind="ExternalInput")
    partial = nc.dram_tensor("partial", [P, 1], fp32, kind="ExternalOutput")

    mult = mybir.AluOpType.mult
    add = mybir.AluOpType.add
    relu = mybir.ActivationFunctionType.Relu
    copyf = mybir.ActivationFunctionType.Copy

    with ExitStack() as ctx:
        tc = ctx.enter_context(tile.TileContext(nc))
        iop = ctx.enter_context(tc.tile_pool(name="iop", bufs=4))
        ifp = ctx.enter_context(tc.tile_pool(name="ifp", bufs=4))
        cvp = ctx.enter_context(tc.tile_pool(name="cvp", bufs=4))
        prp = ctx.enter_context(tc.tile_pool(name="prp", bufs=4))
        psp = ctx.enter_context(tc.psum_pool(name="psp", bufs=4))
        hxp = ctx.enter_context(tc.tile_pool(name="hxp", bufs=4))
        acc = ctx.enter_context(tc.tile_pool(name="acc", bufs=1))

        ones_sb = acc.tile([P, 2 * P], bf16, tag="ones")
        nc.sync.dma_start(out=ones_sb[:], in_=onesc[:])
        pos = ones_sb[:, 0:P]
        neg = ones_sb[:, P:2 * P]

        # 2 hinge columns per (chunk, half); last chunk uses 2 halves
        hsum = acc.tile([P, 2 * (NCH + 1)], fp32, tag="hsum")
        hcol = [0]

        def process(cast_t, raw_t, c0, c1):
            """Compute products/reduce/hinge for rows [c0:c1) of a chunk."""
            w = c1 - c0
            li = cast_t[:, 0:4, c0:c1]
            la = cast_t[:, 4:8, c0:c1]
            gilo = cast_t[:, 8:10, c0:c1]

            gihi_t = cvp.tile([P, 2, CH], bf16, tag="gihi")
            gihi = gihi_t[:, :, c0:c1]
            nc.vector.tensor_copy(out=gihi[:], in_=raw_t[:, 0:2, c0:c1])
            gab_t = cvp.tile([P, 4, CH], bf16, tag="gab")
            gab = gab_t[:, :, c0:c1]
            nc.scalar.activation(out=gab[:], in_=raw_t[:, 2:6, c0:c1],
                                 func=copyf)

            prA_t = prp.tile([P, A, CH], bf16, tag="prA")
            prA = prA_t[:, :, c0:c1]
            nc.vector.tensor_tensor(out=prA[:], in0=li[:], in1=la[:], op=mult)
            prI_t = prp.tile([P, A, CH], bf16, tag="prI")
            prI = prI_t[:, :, c0:c1]
            nc.vector.tensor_tensor(out=prI[:, 0:2], in0=gilo[:],
                                    in1=cast_t[:, 4:6, c0:c1], op=mult)
            nc.vector.tensor_tensor(out=prI[:, 2:4], in0=gihi[:],
                                    in1=cast_t[:, 6:8, c0:c1], op=mult)
            prU_t = prp.tile([P, A, CH], bf16, tag="prU")
            prU = prU_t[:, :, c0:c1]
            nc.vector.tensor_tensor(out=prU[:], in0=li[:], in1=gab[:],
                                    op=mult)

            px_t = psp.tile([P, CH], fp32, tag="px")
            px = px_t[:, c0:c1]
            py_t = psp.tile([P, CH], fp32, tag="py")
            py = py_t[:, c0:c1]
            for a in range(A):
                nc.tensor.matmul(px[:], pos, prI[:, a], start=(a == 0),
                                 stop=False, skip_group_check=True)
            for a in range(A):
                nc.tensor.matmul(py[:], pos, prU[:, a], start=(a == 0),
                                 stop=False, skip_group_check=True)
            for a in range(A):
                nc.tensor.matmul(px[:], neg, prA[:, a], start=False,
                                 stop=(a == A - 1), skip_group_check=True)
            for a in range(A):
                nc.tensor.matmul(py[:], neg, prA[:, a], start=False,
                                 stop=(a == A - 1), skip_group_check=True)

            j = hcol[0]
            hx_t = hxp.tile([P, CH], bf16, tag="hx")
            hx = hx_t[:, c0:c1]
            nc.scalar.activation(out=hx[:], in_=px[:], func=relu, bias=1.0,
                                 scale=1.0, accum_out=hsum[:, j:j + 1])
            hy_t = hxp.tile([P, CH], bf16, tag="hy")
            hy = hy_t[:, c0:c1]
            nc.scalar.activation(out=hy[:], in_=py[:], func=relu, bias=1.0,
                                 scale=1.0, accum_out=hsum[:, j + 1:j + 2])
            hcol[0] = j + 2

        for k in range(NCH):
            cast_t = iop.tile([P, 10, CH], bf16, tag="cast")
            nc.gpsimd.dma_start(out=cast_t[:], in_=dcast[k])  # fp8->bf16
            raw_t = ifp.tile([P, 6, CH], fp8, tag="raw")
            nc.sync.dma_start(out=raw_t[:], in_=draw[k])      # raw fp8
            if k < NCH - 1:
                process(cast_t, raw_t, 0, CH)
            else:
                process(cast_t, raw_t, 0, CH // 2)
                process(cast_t, raw_t, CH // 2, CH)

        psum_t = acc.tile([P, 1], fp32, tag="psum")
        nc.vector.tensor_reduce(
            out=psum_t[:], in_=hsum[:], axis=mybir.AxisListType.X, op=add,
        )
        nc.sync.dma_start(out=partial[:], in_=psum_t[:])

    nc.compile()
    return nc


def _get_nc():
    if "nc" not in _CACHE:
        _CACHE["nc"] = _build_nc()
    return _CACHE["nc"]


def _block(xt):
    """[D, SHARD] -> [NCH, P, A, CH]: per (chunk, partition) contiguous."""
    return np.ascontiguousarray(
        xt.reshape(A, P, NCH, CH).transpose(2, 1, 0, 3))


def make_in_maps(image_outputs, audio_outputs, I_imp_ind, A_imp_ind):
    import ml_dtypes

    bf16 = np.dtype(ml_dtypes.bfloat16)
    fp8 = np.dtype(ml_dtypes.float8_e4m3fn)
    img = np.asarray(image_outputs, dtype=np.float32)
    aud = np.asarray(audio_outputs, dtype=np.float32)
    I_imp = np.asarray(I_imp_ind).astype(np.int64)
    A_imp = np.asarray(A_imp_ind).astype(np.int64)
    ones = np.concatenate(
        [np.ones((P, P), np.float32), -np.ones((P, P), np.float32)],
        axis=1).astype(bf16)
    in_maps = []
    for c in range(NCORES):
        base = c * SHARD
        sl = slice(base, base + SHARD)
        bli = _block(img[sl].T)
        bla = _block(aud[sl].T)
        bgi = _block(img[I_imp[sl]].T)
        bga = _block(aud[A_imp[sl]].T)
        cast = np.concatenate([bli, bla, bgi[:, :, 0:2]], axis=2)
        raw = np.concatenate([bgi[:, :, 2:4], bga], axis=2)
        in_maps.append({
            "dcast": np.ascontiguousarray(cast).astype(fp8),
            "draw": np.ascontiguousarray(raw).astype(fp8),
            "onesc": ones,
        })
    return in_maps


def kernel(image_outputs, audio_outputs, I_imp_ind, A_imp_ind):
    from concourse import bass_utils

    nc = _get_nc()
    in_maps = make_in_maps(image_outputs, audio_outputs, I_imp_ind, A_imp_ind)
    res = bass_utils.run_bass_kernel_spmd(nc, in_maps, list(range(NCORES))).results
    # every PSUM partition holds identical broadcast sums -> use row 0 only
    total = sum(float(r["partial"][0, 0]) for r in res)
    return np.float32(total / N)
